# revision 4
# baseline (speedup 1.0000x reference)
"""Trainium2 Bass kernel for GPLinear (geometric-product linear layer, Cl(3,0)).

    out[b,o,k] = sum_{i,j,p} G[i,j,k] * x[b,p,i] * W[p,o,j] + bias[o,k]

Algorithm: Cl(3,0) is isomorphic to M2(C) via the Pauli matrices
(e_k -> sigma_k).  Multivector v maps to the 2x2 complex matrix
    V = (v0 + i v123) I + (v1 + i v23) s1 + (v2 - i v13) s2 + (v3 + i v12) s3
and the geometric product becomes a plain 2x2 complex matrix product.  The
whole layer is therefore a 2x2-blocked complex matmul:

    C[u,v] = sum_w  A[u,w] @ B[w,v]     (complex [B,P] @ [P,O])

which is 32 real matmuls -- half the 64 blade-matmuls of the direct
XOR-sparse formulation.  The +-1 transforms to/from the matrix rep are done
on the host (cheap), as is the final reduction over the p-shards + bias.

Per core (inputs pre-transformed and pre-transposed on host, fp16):
  - xa[p, 8, b]:   A planes, lhsT layout, u-major: Ar(u,w)=4u+w, Ai=4u+2+w
  - wb[p, 12, o]:  B planes * 0.5, w-major: Br(w,v)=6w+v, Bi=6w+2+v,
                   -Bi=6w+4+v (negated planes realize the complex subtract)
  - 64 matmuls of [128c,128m]x[128c,512n] fp16; each PSUM bank holds one
    accumulation chain = one C-entry row [C(u,0)|C(u,1)] (PSUM
    accumulation groups are 2KB-bank granular); one PSUM tile per bank
    because the tile dependency tracker is tile-granular
  - last p-tile runs all Cre matmuls before all Cim so the Cre
    evacuation overlaps the Cim matmuls
  - evacuation: plain DVE copies PSUM->SBUF fp16, plane-major (GpSimd
    cannot access PSUM; tensor_tensor may read only one PSUM operand, so
    the blade +- combinations happen on the host instead); re-planes DMA
    out early on SP, im-planes on the Activation queue
  - input DMAs chunked in consumption order, split across the SP (xa)
    and Activation (wb) HWDGE queues
  - host sums the two p-shards, combines C planes to blades, adds bias
  - dummy warmup matmuls ramp the PE DVFS p-state during the DMA fill

Sharding (8 cores): 2-way batch x 2-way out_features x 2-way in_features(p).
"""

import numpy as np

import concourse.bass as bass
import concourse.mybir as mybir
import concourse.tile as tile
from concourse import bacc
from concourse.bass_utils import run_bass_kernel_spmd

F32 = mybir.dt.float32
F16 = mybir.dt.float16

BATCH, IN_F, OUT_F, K8 = 512, 512, 512, 8
R_B, R_O, R_P = 2, 2, 2
N_CORES = R_B * R_O * R_P
B_LOC = BATCH // R_B            # 256 batch rows per core
OC = OUT_F // R_O               # 256 out features per core
PC = IN_F // R_P                # 256 contraction rows per core
PT = PC // 128                  # 2 p-tiles
NA = 8                          # A planes
NB = 12                         # B planes
N_WARMUP = 4                    # dummy matmuls to ramp PE p-state

LAST_RESULTS = None


def _check_G(G):
    """Assert G is the exact Cl(3,0) Cayley tensor (bitmask blades)."""
    G = np.asarray(G, dtype=np.float32)
    assert G.shape == (8, 8, 8)
    E = np.zeros((8, 8, 8), dtype=np.float32)
    for a in range(8):
        for b in range(8):
            swaps, t = 0, a >> 1
            while t:
                swaps += bin(t & b).count("1")
                t >>= 1
            E[a, b, a ^ b] = -1.0 if (swaps & 1) else 1.0
    assert np.array_equal(G, E), "G is not the Cl(3,0) Cayley tensor"


def _to_planes(v):
    """v[..., 8] -> planes [8, ...]: entry (u,w) re at 2*(2u+w), im +1."""
    return np.stack([
        v[..., 0] + v[..., 4],   # X00 re
        v[..., 7] + v[..., 3],   # X00 im
        v[..., 1] - v[..., 5],   # X01 re
        v[..., 6] - v[..., 2],   # X01 im
        v[..., 1] + v[..., 5],   # X10 re
        v[..., 6] + v[..., 2],   # X10 im
        v[..., 0] - v[..., 4],   # X11 re
        v[..., 7] - v[..., 3],   # X11 im
    ])


def _build(loop_n=None, unroll=1, staggered=True, warmup=None):
    if warmup is None:
        warmup = N_WARMUP
    nc = bacc.Bacc("TRN2", target_bir_lowering=False, debug=False)

    xa_d = nc.dram_tensor("xa", [PC, NA * B_LOC], F16, kind="ExternalInput")
    wb_d = nc.dram_tensor("wb", [PC, NB * OC], F16, kind="ExternalInput")
    o_d = nc.dram_tensor("out", [B_LOC, OC * K8], F16, kind="ExternalOutput")

    import contextlib

    with tile.TileContext(nc) as tc:
        with (
            tc.tile_pool(name="sb", bufs=1) as sb,
            tc.tile_pool(name="ps", bufs=1, space="PSUM") as ps,
        ):
            nbuf = 2 if (loop_n and unroll == 1) else 1
            ncopy = unroll if loop_n else 1
            xa_t = [sb.tile([128, PT, NA, B_LOC], F16, tag=f"xa{u}",
                            name=f"xa{u}", bufs=nbuf) for u in range(ncopy)]
            wb_t = [sb.tile([128, PT, NB, OC], F16, tag=f"wb{u}",
                            name=f"wb{u}", bufs=nbuf) for u in range(ncopy)]
            out_t = [[[sb.tile([128, 4 * OC], F16, tag=f"out{u}_{bt}{g}",
                               name=f"out{u}_{bt}{g}", bufs=nbuf)
                       for g in range(2)]
                      for bt in range(2)] for u in range(ncopy)]
            # one PSUM tile per bank: dependency tracking is
            # tile-granular, so per-bank tiles let each consumer start the
            # moment its own accumulation chain stops.
            # acc_re[bt][u] holds planes [Cre(u,0)|Cre(u,1)]; _im likewise.
            acc_re = [[ps.tile([128, 2 * OC], F32, tag=f"accre{bt}{u}",
                               name=f"accre{bt}{u}") for u in range(2)]
                      for bt in range(2)]
            acc_im = [[ps.tile([128, 2 * OC], F32, tag=f"accim{bt}{u}",
                               name=f"accim{bt}{u}") for u in range(2)]
                      for bt in range(2)]
            dum = sb.tile([128, 2 * OC], F16, tag="dum")
            nc.gpsimd.memzero(dum[:])

            pitch_p = acc_re[0][0][:].ap[0][0]

            def mm(bt, kind, u, w, pt, ar_side, start=False, stop=False):
                # kind: 'cre' (Ar@Br | Ai@-Bi) or 'cim' (Ar@Bi | Ai@Br)
                xa, wb = mm.xa, mm.wb
                bs = slice(bt * 128, (bt + 1) * 128)
                lhs = xa[:, pt, 4 * u + (0 if ar_side else 2) + w, bs]
                if kind == "cre":
                    out = acc_re[bt][u][:]
                    rhs = (wb[:, pt, 6 * w:6 * w + 2, :] if ar_side
                           else wb[:, pt, 6 * w + 4:6 * w + 6, :])
                else:
                    out = acc_im[bt][u][:]
                    rhs = (wb[:, pt, 6 * w + 2:6 * w + 4, :] if ar_side
                           else wb[:, pt, 6 * w:6 * w + 2, :])
                nc.tensor.matmul(out, lhs, rhs, start=start, stop=stop)

            def evac(bt, out_sb, group, eng0, eng1):
                # plain PSUM->SBUF copies, plane-major fp16; the blade
                # +- combinations happen on the host during the p-shard
                # reduction (tensor_tensor may read only one PSUM operand,
                # so combining here would need an extra staging hop).
                accg = (acc_re if group == "cre" else acc_im)[bt]
                dst = out_sb[0 if group == "cre" else 1]
                for eng, half, a in ((eng0, 0, accg[0]), (eng1, 1, accg[1])):
                    if eng is nc.scalar:
                        eng.copy(dst[:, half * 2 * OC:(half + 1) * 2 * OC],
                                 a[:])
                    else:
                        eng.tensor_copy(
                            dst[:, half * 2 * OC:(half + 1) * 2 * OC], a[:])

            def body(xa, wb, out_sb):
                mm.xa, mm.wb = xa, wb
                if warmup:
                    # ramp PE DVFS while the input DMAs land
                    for i in range(warmup):
                        nc.tensor.matmul(
                            acc_re[0][0][:], dum[:, :128], dum[:],
                            start=(i == 0), stop=(i == warmup - 1),
                            skip_group_check=True)

                # chunked input DMAs in consumption order, split across
                # the SP (xa) and Activation (wb) HWDGE queues
                for pt in range(PT):
                    if True:
                        for h in range(2):
                            nc.sync.dma_start(
                                xa[:, pt, 4 * h:4 * h + 4, :],
                                xa_d.ap()[pt * 128:(pt + 1) * 128,
                                          4 * h * B_LOC:(4 * h + 4) * B_LOC]
                                    .rearrange("p (e b) -> p e b", e=4))
                            nc.scalar.dma_start(
                                wb[:, pt, 6 * h:6 * h + 6, :],
                                wb_d.ap()[pt * 128:(pt + 1) * 128,
                                          6 * h * OC:(6 * h + 6) * OC]
                                    .rearrange("p (j o) -> p j o", j=6))


                for bt in range(2):
                    # p-tiles 0..PT-2: all 16 matmuls, u-major
                    for pt in range(PT - 1):
                        for u in range(2):
                            for w in range(2):
                                st = (pt == 0 and w == 0)
                                mm(bt, "cre", u, w, pt, True, start=st)
                                mm(bt, "cim", u, w, pt, True, start=st)
                                mm(bt, "cre", u, w, pt, False)
                                mm(bt, "cim", u, w, pt, False)
                    # last p-tile: Cre pass then Cim pass, so Cre banks
                    # stop early and their evacuation overlaps Cim
                    # matmuls; u=1 first so the Act staging copy (reads
                    # the u=1 bank) starts earliest.
                    for u in (1, 0):
                        for w in range(2):
                            mm(bt, "cre", u, w, PT - 1, True)
                            mm(bt, "cre", u, w, PT - 1, False,
                               stop=(w == 1))
                    evac(bt, out_sb[bt], "cre", nc.vector, nc.vector)
                    nc.sync.dma_start(
                        o_d.ap()[bt * 128:(bt + 1) * 128, 0:4 * OC],
                        out_sb[bt][0][:])
                    for u in (1, 0):
                        for w in range(2):
                            mm(bt, "cim", u, w, PT - 1, True)
                            mm(bt, "cim", u, w, PT - 1, False,
                               stop=(w == 1))
                    evac(bt, out_sb[bt], "cim", nc.vector, nc.vector)
                    # im-planes ride the Activation queue: its wb DMAs are
                    # done by now, while SP is still busy with the re DMA
                    nc.scalar.dma_start(
                        o_d.ap()[bt * 128:(bt + 1) * 128, 4 * OC:8 * OC],
                        out_sb[bt][1][:])

            with (tc.For_i(0, loop_n, 1, staggered_reset=staggered)
                  if loop_n else contextlib.nullcontext()):
                for u in range(ncopy):
                    body(xa_t[u], wb_t[u], out_t[u])

    nc.compile()
    return nc


_CACHE = {}


def _prep_inputs(x, W):
    """Host-side transform to fp16 plane stacks, per core."""
    A = _to_planes(np.asarray(x, dtype=np.float32))        # [8, B, P]
    Bp = _to_planes(np.asarray(W, dtype=np.float32)) * 0.5  # [8, P, O]
    # xa u-major: [Ar00,Ar01,Ai00,Ai01,Ar10,Ar11,Ai10,Ai11]
    xa = np.ascontiguousarray(
        np.transpose(A[[0, 2, 1, 3, 4, 6, 5, 7]], (2, 0, 1)),
        dtype=np.float16)                                   # [P, 8, B]
    # wb w-major groups: [Br(w,0),Br(w,1),Bi(w,0),Bi(w,1),-Bi(w,0),-Bi(w,1)]
    wb = np.ascontiguousarray(
        np.transpose(np.stack([
            Bp[0], Bp[2], Bp[1], Bp[3], -Bp[1], -Bp[3],
            Bp[4], Bp[6], Bp[5], Bp[7], -Bp[5], -Bp[7],
        ]), (1, 0, 2)), dtype=np.float16)                   # [P, 12, O]
    in_maps = []
    for c in range(N_CORES):
        pc, r = divmod(c, R_B * R_O)
        bc, oc = divmod(r, R_O)
        in_maps.append({
            "xa": np.ascontiguousarray(
                xa[pc * PC:(pc + 1) * PC, :, bc * B_LOC:(bc + 1) * B_LOC]
                ).reshape(PC, NA * B_LOC),
            "wb": np.ascontiguousarray(
                wb[pc * PC:(pc + 1) * PC, :, oc * OC:(oc + 1) * OC]
                ).reshape(PC, NB * OC),
        })
    return in_maps


def _combine_out(raw):
    """Device C planes [b, plane, o] -> blades [b, o, 8] (f32).

    plane order: Cre00,Cre01,Cre10,Cre11,Cim00,Cim01,Cim10,Cim11."""
    P = np.asarray(raw).reshape(B_LOC, 8, OC).astype(np.float32)
    o = np.empty((B_LOC, OC, 8), dtype=np.float32)
    o[:, :, 0] = P[:, 0] + P[:, 3]
    o[:, :, 1] = P[:, 1] + P[:, 2]
    o[:, :, 4] = P[:, 0] - P[:, 3]
    o[:, :, 5] = P[:, 2] - P[:, 1]
    o[:, :, 7] = P[:, 4] + P[:, 7]
    o[:, :, 6] = P[:, 5] + P[:, 6]
    o[:, :, 3] = P[:, 4] - P[:, 7]
    o[:, :, 2] = P[:, 6] - P[:, 5]
    return o


def kernel(x, W, b, G):
    global LAST_RESULTS
    _check_G(G)

    if "nc" not in _CACHE:
        _CACHE["nc"] = _build()
    nc = _CACHE["nc"]

    in_maps = _prep_inputs(x, W)
    res = run_bass_kernel_spmd(nc, in_maps, core_ids=list(range(N_CORES)))
    LAST_RESULTS = res

    out = np.zeros((BATCH, OUT_F, K8), dtype=np.float32)
    for c in range(N_CORES):
        pc, r = divmod(c, R_B * R_O)
        bc, oc = divmod(r, R_O)
        out[bc * B_LOC:(bc + 1) * B_LOC, oc * OC:(oc + 1) * OC, :] += \
            _combine_out(res.results[c]["out"])
    out += np.asarray(b, dtype=np.float32)[None]
    return out


# revision 5
# speedup vs baseline: 1.0938x; 1.0938x over previous
"""Trainium2 Bass kernel for GPLinear (geometric-product linear layer, Cl(3,0)).

    out[b,o,k] = sum_{i,j,p} G[i,j,k] * x[b,p,i] * W[p,o,j] + bias[o,k]

Algorithm: Cl(3,0) is isomorphic to M2(C) via the Pauli matrices
(e_k -> sigma_k).  Multivector v maps to the 2x2 complex matrix
    V = (v0 + i v123) I + (v1 + i v23) s1 + (v2 - i v13) s2 + (v3 + i v12) s3
and the geometric product becomes a plain 2x2 complex matrix product.  The
whole layer is therefore a 2x2-blocked complex matmul:

    C[u,v] = sum_w  A[u,w] @ B[w,v]     (complex [B,P] @ [P,O])

which is 32 real matmuls -- half the 64 blade-matmuls of the direct
XOR-sparse formulation.  The +-1 transforms to/from the matrix rep are done
on the host (cheap), as is the final reduction over the p-shards + bias.

Per core (inputs pre-transformed and pre-transposed on host, fp16):
  - xa[p, 8, b]:   A planes, lhsT layout, u-major: Ar(u,w)=4u+w, Ai=4u+2+w
  - wb[p, 12, o]:  B planes * 0.5, w-major: Br(w,v)=6w+v, Bi=6w+2+v,
                   -Bi=6w+4+v (negated planes realize the complex subtract)
  - 64 matmuls of [128c,128m]x[128c,512n] fp16; each PSUM bank holds one
    accumulation chain = one C-entry row [C(u,0)|C(u,1)] (PSUM
    accumulation groups are 2KB-bank granular); one PSUM tile per bank
    because the tile dependency tracker is tile-granular
  - last p-tile runs all Cre matmuls before all Cim so the Cre
    evacuation overlaps the Cim matmuls
  - evacuation: plain DVE copies PSUM->SBUF fp16, plane-major (GpSimd
    cannot access PSUM; tensor_tensor may read only one PSUM operand, so
    the blade +- combinations happen on the host instead); re-planes DMA
    out early on SP, im-planes on the Activation queue
  - input DMAs chunked in consumption order, split across the SP (xa)
    and Activation (wb) HWDGE queues
  - host sums the two p-shards, combines C planes to blades, adds bias
  - dummy warmup matmuls ramp the PE DVFS p-state during the DMA fill

Sharding (8 cores): 2-way batch x 2-way out_features x 2-way in_features(p).
"""

import numpy as np

import concourse.bass as bass
import concourse.mybir as mybir
import concourse.tile as tile
from concourse import bacc
from concourse.bass_utils import run_bass_kernel_spmd

F32 = mybir.dt.float32
F16 = mybir.dt.float16

BATCH, IN_F, OUT_F, K8 = 512, 512, 512, 8
R_B, R_O, R_P = 2, 2, 2
N_CORES = R_B * R_O * R_P
B_LOC = BATCH // R_B            # 256 batch rows per core
OC = OUT_F // R_O               # 256 out features per core
PC = IN_F // R_P                # 256 contraction rows per core
PT = PC // 128                  # 2 p-tiles
NA = 8                          # A planes
NB = 12                         # B planes
N_WARMUP = 5                    # dummy matmuls to ramp PE p-state

LAST_RESULTS = None


def _check_G(G):
    """Assert G is the exact Cl(3,0) Cayley tensor (bitmask blades)."""
    G = np.asarray(G, dtype=np.float32)
    assert G.shape == (8, 8, 8)
    E = np.zeros((8, 8, 8), dtype=np.float32)
    for a in range(8):
        for b in range(8):
            swaps, t = 0, a >> 1
            while t:
                swaps += bin(t & b).count("1")
                t >>= 1
            E[a, b, a ^ b] = -1.0 if (swaps & 1) else 1.0
    assert np.array_equal(G, E), "G is not the Cl(3,0) Cayley tensor"


def _to_planes(v):
    """v[..., 8] -> planes [8, ...]: entry (u,w) re at 2*(2u+w), im +1."""
    return np.stack([
        v[..., 0] + v[..., 4],   # X00 re
        v[..., 7] + v[..., 3],   # X00 im
        v[..., 1] - v[..., 5],   # X01 re
        v[..., 6] - v[..., 2],   # X01 im
        v[..., 1] + v[..., 5],   # X10 re
        v[..., 6] + v[..., 2],   # X10 im
        v[..., 0] - v[..., 4],   # X11 re
        v[..., 7] - v[..., 3],   # X11 im
    ])


def _build(loop_n=None, unroll=1, staggered=True, warmup=None):
    if warmup is None:
        warmup = N_WARMUP
    nc = bacc.Bacc("TRN2", target_bir_lowering=False, debug=False)

    xa_d = nc.dram_tensor("xa", [PC, NA * B_LOC], F16, kind="ExternalInput")
    wb_d = nc.dram_tensor("wb", [PC, NB * OC], F16, kind="ExternalInput")
    o_d = nc.dram_tensor("out", [B_LOC, OC * K8], F16, kind="ExternalOutput")

    import contextlib

    with tile.TileContext(nc) as tc:
        with (
            tc.tile_pool(name="sb", bufs=1) as sb,
            tc.tile_pool(name="ps", bufs=1, space="PSUM") as ps,
        ):
            nbuf = 2 if (loop_n and unroll == 1) else 1
            ncopy = unroll if loop_n else 1
            xa_t = [sb.tile([128, PT, NA, B_LOC], F16, tag=f"xa{u}",
                            name=f"xa{u}", bufs=nbuf) for u in range(ncopy)]
            wb_t = [sb.tile([128, PT, NB, OC], F16, tag=f"wb{u}",
                            name=f"wb{u}", bufs=nbuf) for u in range(ncopy)]
            out_t = [[[sb.tile([128, 4 * OC], F16, tag=f"out{u}_{bt}{g}",
                               name=f"out{u}_{bt}{g}", bufs=nbuf)
                       for g in range(2)]
                      for bt in range(2)] for u in range(ncopy)]
            # one PSUM tile per bank: dependency tracking is
            # tile-granular, so per-bank tiles let each consumer start the
            # moment its own accumulation chain stops.
            # acc_re[bt][u] holds planes [Cre(u,0)|Cre(u,1)]; _im likewise.
            acc_re = [[ps.tile([128, 2 * OC], F32, tag=f"accre{bt}{u}",
                               name=f"accre{bt}{u}") for u in range(2)]
                      for bt in range(2)]
            acc_im = [[ps.tile([128, 2 * OC], F32, tag=f"accim{bt}{u}",
                               name=f"accim{bt}{u}") for u in range(2)]
                      for bt in range(2)]
            dum = sb.tile([128, 2 * OC], F16, tag="dum")
            nc.gpsimd.memzero(dum[:])

            pitch_p = acc_re[0][0][:].ap[0][0]

            def mm(bt, kind, u, w, pt, ar_side, start=False, stop=False):
                # kind: 'cre' (Ar@Br | Ai@-Bi) or 'cim' (Ar@Bi | Ai@Br)
                xa, wb = mm.xa, mm.wb
                bs = slice(bt * 128, (bt + 1) * 128)
                lhs = xa[:, pt, 4 * u + (0 if ar_side else 2) + w, bs]
                if kind == "cre":
                    out = acc_re[bt][u][:]
                    rhs = (wb[:, pt, 6 * w:6 * w + 2, :] if ar_side
                           else wb[:, pt, 6 * w + 4:6 * w + 6, :])
                else:
                    out = acc_im[bt][u][:]
                    rhs = (wb[:, pt, 6 * w + 2:6 * w + 4, :] if ar_side
                           else wb[:, pt, 6 * w:6 * w + 2, :])
                nc.tensor.matmul(out, lhs, rhs, start=start, stop=stop)

            def evac(bt, out_sb, group, eng0, eng1):
                # plain PSUM->SBUF copies, plane-major fp16; the blade
                # +- combinations happen on the host during the p-shard
                # reduction (tensor_tensor may read only one PSUM operand,
                # so combining here would need an extra staging hop).
                accg = (acc_re if group == "cre" else acc_im)[bt]
                dst = out_sb[0 if group == "cre" else 1]
                for eng, half, a in ((eng0, 0, accg[0]), (eng1, 1, accg[1])):
                    if eng is nc.scalar:
                        eng.copy(dst[:, half * 2 * OC:(half + 1) * 2 * OC],
                                 a[:])
                    else:
                        eng.tensor_copy(
                            dst[:, half * 2 * OC:(half + 1) * 2 * OC], a[:])

            def body(xa, wb, out_sb):
                mm.xa, mm.wb = xa, wb
                if warmup:
                    # ramp PE DVFS while the input DMAs land
                    for i in range(warmup):
                        nc.tensor.matmul(
                            acc_re[0][0][:], dum[:, :128], dum[:],
                            start=(i == 0), stop=(i == warmup - 1),
                            skip_group_check=True)

                # chunked input DMAs in consumption order, split across
                # the SP (xa) and Activation (wb) HWDGE queues
                for pt in range(PT):
                    if True:
                        for h in range(2):
                            nc.sync.dma_start(
                                xa[:, pt, 4 * h:4 * h + 4, :],
                                xa_d.ap()[pt * 128:(pt + 1) * 128,
                                          4 * h * B_LOC:(4 * h + 4) * B_LOC]
                                    .rearrange("p (e b) -> p e b", e=4))
                            nc.scalar.dma_start(
                                wb[:, pt, 6 * h:6 * h + 6, :],
                                wb_d.ap()[pt * 128:(pt + 1) * 128,
                                          6 * h * OC:(6 * h + 6) * OC]
                                    .rearrange("p (j o) -> p j o", j=6))


                for bt in range(2):
                    # p-tiles 0..PT-2: all 16 matmuls, u-major
                    for pt in range(PT - 1):
                        for u in range(2):
                            for w in range(2):
                                st = (pt == 0 and w == 0)
                                mm(bt, "cre", u, w, pt, True, start=st)
                                mm(bt, "cim", u, w, pt, True, start=st)
                                mm(bt, "cre", u, w, pt, False)
                                mm(bt, "cim", u, w, pt, False)
                    # last p-tile: Cre pass then Cim pass, so Cre banks
                    # stop early and their evacuation overlaps Cim
                    # matmuls; u=1 first so the Act staging copy (reads
                    # the u=1 bank) starts earliest.
                    for u in (1, 0):
                        for w in range(2):
                            mm(bt, "cre", u, w, PT - 1, True)
                            mm(bt, "cre", u, w, PT - 1, False,
                               stop=(w == 1))
                    evac(bt, out_sb[bt], "cre", nc.vector, nc.vector)
                    nc.sync.dma_start(
                        o_d.ap()[bt * 128:(bt + 1) * 128, 0:4 * OC],
                        out_sb[bt][0][:])
                    for u in (1, 0):
                        for w in range(2):
                            mm(bt, "cim", u, w, PT - 1, True)
                            mm(bt, "cim", u, w, PT - 1, False,
                               stop=(w == 1))
                    evac(bt, out_sb[bt], "cim", nc.vector, nc.vector)
                    # im-planes ride the Activation queue: its wb DMAs are
                    # done by now, while SP is still busy with the re DMA
                    nc.scalar.dma_start(
                        o_d.ap()[bt * 128:(bt + 1) * 128, 4 * OC:8 * OC],
                        out_sb[bt][1][:])

            with (tc.For_i(0, loop_n, 1, staggered_reset=staggered)
                  if loop_n else contextlib.nullcontext()):
                for u in range(ncopy):
                    body(xa_t[u], wb_t[u], out_t[u])

    nc.compile()
    return nc


_CACHE = {}


def _prep_inputs(x, W):
    """Host-side transform to fp16 plane stacks, per core."""
    A = _to_planes(np.asarray(x, dtype=np.float32))        # [8, B, P]
    Bp = _to_planes(np.asarray(W, dtype=np.float32)) * 0.5  # [8, P, O]
    # xa u-major: [Ar00,Ar01,Ai00,Ai01,Ar10,Ar11,Ai10,Ai11]
    xa = np.ascontiguousarray(
        np.transpose(A[[0, 2, 1, 3, 4, 6, 5, 7]], (2, 0, 1)),
        dtype=np.float16)                                   # [P, 8, B]
    # wb w-major groups: [Br(w,0),Br(w,1),Bi(w,0),Bi(w,1),-Bi(w,0),-Bi(w,1)]
    wb = np.ascontiguousarray(
        np.transpose(np.stack([
            Bp[0], Bp[2], Bp[1], Bp[3], -Bp[1], -Bp[3],
            Bp[4], Bp[6], Bp[5], Bp[7], -Bp[5], -Bp[7],
        ]), (1, 0, 2)), dtype=np.float16)                   # [P, 12, O]
    in_maps = []
    for c in range(N_CORES):
        pc, r = divmod(c, R_B * R_O)
        bc, oc = divmod(r, R_O)
        in_maps.append({
            "xa": np.ascontiguousarray(
                xa[pc * PC:(pc + 1) * PC, :, bc * B_LOC:(bc + 1) * B_LOC]
                ).reshape(PC, NA * B_LOC),
            "wb": np.ascontiguousarray(
                wb[pc * PC:(pc + 1) * PC, :, oc * OC:(oc + 1) * OC]
                ).reshape(PC, NB * OC),
        })
    return in_maps


def _combine_out(raw):
    """Device C planes [b, plane, o] -> blades [b, o, 8] (f32).

    plane order: Cre00,Cre01,Cre10,Cre11,Cim00,Cim01,Cim10,Cim11."""
    P = np.asarray(raw).reshape(B_LOC, 8, OC).astype(np.float32)
    o = np.empty((B_LOC, OC, 8), dtype=np.float32)
    o[:, :, 0] = P[:, 0] + P[:, 3]
    o[:, :, 1] = P[:, 1] + P[:, 2]
    o[:, :, 4] = P[:, 0] - P[:, 3]
    o[:, :, 5] = P[:, 2] - P[:, 1]
    o[:, :, 7] = P[:, 4] + P[:, 7]
    o[:, :, 6] = P[:, 5] + P[:, 6]
    o[:, :, 3] = P[:, 4] - P[:, 7]
    o[:, :, 2] = P[:, 6] - P[:, 5]
    return o


def kernel(x, W, b, G):
    global LAST_RESULTS
    _check_G(G)

    if "nc" not in _CACHE:
        _CACHE["nc"] = _build()
    nc = _CACHE["nc"]

    in_maps = _prep_inputs(x, W)
    res = run_bass_kernel_spmd(nc, in_maps, core_ids=list(range(N_CORES)))
    LAST_RESULTS = res

    out = np.zeros((BATCH, OUT_F, K8), dtype=np.float32)
    for c in range(N_CORES):
        pc, r = divmod(c, R_B * R_O)
        bc, oc = divmod(r, R_O)
        out[bc * B_LOC:(bc + 1) * B_LOC, oc * OC:(oc + 1) * OC, :] += \
            _combine_out(res.results[c]["out"])
    out += np.asarray(b, dtype=np.float32)[None]
    return out
